# revision 13
# baseline (speedup 1.0000x reference)
"""Trainium2 Bass kernel for nn_Canvas_DIP_by_distance (vq_codebook), v2.

reference semantics:
  weight = sigmoid(weight_logits)                       (224, 224, 3)
  d[h,w,c] = sum_k (palette[c,k] - weight[h,w,k])^2     (224, 224, 64)
  idx = argmax_c softmax(d + 1) = argmax_c d
  colors[ch,h,w] = palette[idx[h,w], ch]                (3, 224, 224)
  out = nearest_upsample(colors, 2048, 2048)            (3, 2048, 2048)

v2 design (per core: 28 canvas rows -> 256 output rows):
  - host precomputes sigmoid + the w-major (h,k)-transposed layout, so the
    device does no sigmoid and no strided input DMA (fat descriptors only).
  - v[w,(j,c)] via ONE block-diagonal fp32 matmul per (quarter, w-half):
    lhsT = w4g [28=(7j 4k), 112w], rhs = b4c [28, 448=(7j 64c)].
  - argmax one-hot via reduce_max + is_equal (fp32 exact, baseline-proven).
  - palette apply via 8x8 index factorization: c = 8a + b.
      oha[w,j,a] = max_b oh,   ohb[w,j,b] = max_a oh        (2 DVE reduces)
      ohaT via ONE small PE transpose per (quarter, half)
      M1[w,(j,ch,b)] = sum_a ohaT * P2E (block-diag)        (1 matmul)
      colors[w,j,ch] = sum_b M1 * ohb                       (DVE mult+reduce)
    Output colors are fp16 palette values (exact selects); final error vs
    fp32 palette is <= 2^-11 ~ 5e-4, far under the 2e-2 gate.
  - column expansion: colors [112, 112slots] @ esb -> exp [112slots, 512] x4.
  - row replication: 0/1 RT matmuls [112slots -> 128 output rows] per
    (ch, row-half, col-chunk), PSUM->SBUF copy, then store DMA per chunk.
  - rows 0..127 only need canvas rows hh<=13 (quarters 0,1), so the first
    half of stores streams while quarters 2,3 still compute.

slot layout: slot = 28g + 4j + ch for canvas row hh = 7g + j, channel ch
(the 4j+3 slots stay zero) -- keeps store-side DMA partitions spread.
"""

import numpy as np
from contextlib import ExitStack

CANVAS_H, CANVAS_W, NUM_COLORS = 224, 224, 64
IMAGE_H = IMAGE_W = 2048
N_CORES = 8
HC = CANVAS_H // N_CORES          # 28 canvas rows per core
ORC = IMAGE_H // N_CORES          # 256 output rows per core
WH = CANVAS_W // 2                # 112

_CACHE = {}


def _build_program():
    import concourse.bacc as bacc
    import concourse.tile as tile
    import concourse.mybir as mybir
    from concourse import bass

    f32 = mybir.dt.float32
    f16 = mybir.dt.float16
    ALU = mybir.AluOpType
    nc = bacc.Bacc("TRN2", target_bir_lowering=False)

    w4g_in = nc.dram_tensor("w4g_in", [28, 2, 4, 112], f32, kind="ExternalInput")
    b4c_in = nc.dram_tensor("b4c_in", [28, 448], f32, kind="ExternalInput")
    p2e_in = nc.dram_tensor("p2e_in", [56, 168], f16, kind="ExternalInput")
    id16_in = nc.dram_tensor("id16_in", [112, 112], f16, kind="ExternalInput")
    esb_in = nc.dram_tensor("esb_in", [112, 2, 2048], f16, kind="ExternalInput")
    rt_in = nc.dram_tensor("rt_in", [112, 6, 128], f16, kind="ExternalInput")
    out = nc.dram_tensor("out", [3, ORC, IMAGE_W], f32, kind="ExternalOutput")

    with tile.TileContext(nc) as tc:
        with ExitStack() as ctx:
            sb = ctx.enter_context(tc.tile_pool(name="sb", bufs=1))
            ps = ctx.enter_context(tc.tile_pool(name="ps", bufs=1, space="PSUM"))

            # ---- const loads: small/early on sync, big on scalar ring ----
            w4g = sb.tile([28, 2, 4, 112], f32, tag="w4g")
            nc.sync.dma_start(out=w4g[:], in_=w4g_in[:])
            b4c = sb.tile([28, 448], f32, tag="b4c")
            nc.sync.dma_start(out=b4c[:], in_=b4c_in[:])
            p2e = sb.tile([56, 168], f16, tag="p2e")
            nc.sync.dma_start(out=p2e[:], in_=p2e_in[:])
            id16 = sb.tile([112, 112], f16, tag="id16")
            nc.sync.dma_start(out=id16[:], in_=id16_in[:])
            esb = sb.tile([112, 2, 2048], f16, tag="esb")
            for wf in range(2):
                nc.scalar.dma_start(out=esb[:, wf], in_=esb_in[:, wf])
            rt = sb.tile([112, 6, 128], f16, tag="rt")
            nc.scalar.dma_start(out=rt[:], in_=rt_in[:])

            colors = sb.tile([112, 2, 112], f16, tag="colors")
            nc.vector.memset(colors[:], 0.0)

            exp16 = sb.tile([112, 4, 512], f16, tag="exp16")
            ofs = sb.tile([128, 24, 512], f32, tag="ofs")

            def front(q):
                """quarter q (canvas rows 7q..7q+6), both w-halves -> colors."""
                vq = ps.tile([112, 2, 512], f32, tag="vps", bufs=1)
                for wf in range(2):
                    nc.tensor.matmul(
                        out=vq[:, wf, 0:448], lhsT=w4g[:, wf, q],
                        rhs=b4c[:], start=True, stop=True)
                vv = vq[:, :, 0:448].rearrange(
                    "w f (j a b) -> w f j a b", a=8, b=8)
                m8a = sb.tile([112, 2, 7, 8], f32, tag="m8a", bufs=2)
                nc.vector.tensor_reduce(
                    out=m8a[:], in_=vv, axis=mybir.AxisListType.X, op=ALU.max)
                m8b = sb.tile([112, 2, 7, 8], f32, tag="m8b", bufs=2)
                nc.vector.tensor_reduce(
                    out=m8b[:],
                    in_=vq[:, :, 0:448].rearrange(
                        "w f (j a b) -> w f j b a", a=8, b=8),
                    axis=mybir.AxisListType.X, op=ALU.max)
                vmax = sb.tile([112, 2, 7], f32, tag="vmax", bufs=2)
                nc.vector.tensor_reduce(
                    out=vmax[:], in_=m8a[:], axis=mybir.AxisListType.X,
                    op=ALU.max)
                vmb = vmax[:].unsqueeze(3).to_broadcast([112, 2, 7, 8])
                oha = sb.tile([112, 2, 7, 8], f16, tag="oha", bufs=2)
                nc.vector.tensor_tensor(
                    out=oha[:], in0=m8a[:], in1=vmb, op=ALU.is_equal)
                ohb = sb.tile([112, 2, 7, 8], f16, tag="ohb", bufs=2)
                nc.vector.tensor_tensor(
                    out=ohb[:], in0=m8b[:], in1=vmb, op=ALU.is_equal)
                m1 = ps.tile([112, 2, 256], f32, tag="m1ps", bufs=1)
                for wf in range(2):
                    tps = ps.tile([56, 112], f16, tag="tps", bufs=1)
                    nc.tensor.transpose(
                        out=tps[:],
                        in_=oha[:, wf].rearrange("w j a -> w (j a)"),
                        identity=id16[:, 0:112])
                    ohaT = sb.tile([56, 112], f16, tag="ohaT", bufs=2)
                    nc.scalar.copy(out=ohaT[:], in_=tps[:])
                    nc.tensor.matmul(
                        out=m1[:, wf, 0:168], lhsT=ohaT[:], rhs=p2e[:],
                        start=True, stop=True)
                tmp = sb.tile([112, 2, 7, 3, 8], f16, tag="tmp", bufs=2)
                nc.vector.tensor_tensor(
                    out=tmp[:],
                    in0=m1[:, :, 0:168].rearrange(
                        "w f (j c b) -> w f j c b", c=3, b=8),
                    in1=ohb[:].unsqueeze(3).to_broadcast([112, 2, 7, 3, 8]),
                    op=ALU.mult)
                cdst = (colors[:, :, 28 * q:28 * q + 28]
                        .rearrange("w f (j s) -> w f j s", s=4)[:, :, :, 0:3])
                with nc.allow_low_precision(
                        reason="one-hot select: sum has a single nonzero f16"):
                    nc.vector.tensor_reduce(
                        out=cdst, in_=tmp[:], axis=mybir.AxisListType.X,
                        op=ALU.add)

            def expand(cc):
                """column-expand chunk cc: exp16[:, cc] = colors @ esb."""
                eps = ps.tile([112, 512], f32, tag="eps", bufs=2)
                for wf in range(2):
                    nc.tensor.matmul(
                        out=eps[:], lhsT=colors[:, wf],
                        rhs=esb[:, wf, 512 * cc:512 * cc + 512],
                        start=(wf == 0), stop=(wf == 1))
                eng = nc.vector if (cc % 2 == 0) else nc.scalar
                if eng is nc.vector:
                    eng.tensor_copy(out=exp16[:, cc], in_=eps[:])
                else:
                    eng.copy(out=exp16[:, cc], in_=eps[:])

            def replicate_store(hf2):
                """row-replicate + store output rows 128*hf2 .. +128."""
                for ch in range(3):
                    idx = 2 * ch + hf2
                    for cc in range(4):
                        ops = ps.tile([128, 512], f32, tag="ops", bufs=2)
                        nc.tensor.matmul(
                            out=ops[:], lhsT=rt[:, idx], rhs=exp16[:, cc],
                            start=True, stop=True)
                        oslice = ofs[:, 4 * idx + cc]
                        eng = nc.scalar if (cc % 2 == 0) else nc.vector
                        if eng is nc.vector:
                            eng.tensor_copy(out=oslice, in_=ops[:])
                        else:
                            eng.copy(out=oslice, in_=ops[:])
                        dma = nc.sync if ((ch + cc) % 2 == 0) else nc.scalar
                        dma.dma_start(
                            out=out[ch, 128 * hf2:128 * hf2 + 128,
                                    512 * cc:512 * cc + 512],
                            in_=oslice)

            # quarters 0,1 -> first 128 output rows stream out while 2,3 run
            front(0)
            front(1)
            for cc in range(4):
                expand(cc)
            replicate_store(0)
            front(2)
            front(3)
            for cc in range(4):
                expand(cc)
            replicate_store(1)

    nc.compile()
    return nc


def _host_consts(weight_logits: np.ndarray, palette: np.ndarray):
    """Build per-core input tensors (host does sigmoid + layouts)."""
    pal = palette.astype(np.float32)
    pal16 = pal.astype(np.float16)
    sig = (1.0 / (1.0 + np.exp(-weight_logits.astype(np.float64))))
    sig = sig.astype(np.float32)                      # (224, 224, 3)

    # b4c [28=(7j 4k), 448=(7j 64c)] block-diagonal
    b4row = np.empty((4, NUM_COLORS), np.float32)
    b4row[0:3] = -pal.T
    b4row[3] = 0.5 * (pal.astype(np.float64) ** 2).sum(-1).astype(np.float32)
    b4c = np.zeros((28, 448), np.float32)
    for j in range(7):
        b4c[4 * j:4 * j + 4, 64 * j:64 * j + 64] = b4row

    # p2e [56=(7j 8a), 168=(7j 3ch 8b)] block-diagonal
    p2 = pal16.reshape(8, 8, 3)                       # [a, b, ch]
    blk = np.transpose(p2, (0, 2, 1)).reshape(8, 24)  # [a, (ch b)]
    p2e = np.zeros((56, 168), np.float16)
    for j in range(7):
        p2e[8 * j:8 * j + 8, 24 * j:24 * j + 24] = blk

    # esb [112, 2, 2048] 0/1 column-expansion
    wmap = (np.arange(IMAGE_W) * CANVAS_W) // IMAGE_W
    e_full = (wmap[None, :] == np.arange(CANVAS_W)[:, None]).astype(np.float16)
    esb = np.ascontiguousarray(
        np.stack([e_full[:WH], e_full[WH:]], axis=1))  # (112, 2, 2048)

    # rt [112, 6=(2ch+hf2... idx=2ch+hf2), 128] 0/1 row replication
    rt = np.zeros((112, 6, 128), np.float16)
    for hf2 in range(2):
        for p in range(128):
            r = 128 * hf2 + p
            hh = (r * 7) // 64
            g, j = hh // 7, hh % 7
            for ch in range(3):
                rt[28 * g + 4 * j + ch, 2 * ch + hf2, p] = 1.0

    id16 = np.eye(112, dtype=np.float16)

    # per-core w4g [112=(4q 7j 4k... 28q+4j+k), 2, 112]
    w4gs = []
    for core in range(N_CORES):
        s = sig[core * HC:(core + 1) * HC]            # (28, 224, 3)
        w4g = np.empty((28, 2, 4, 112), np.float32)
        for q in range(4):
            for j in range(7):
                row = s[7 * q + j]                    # (224, 3)
                for k in range(4):
                    v = (row[:, k] if k < 3
                         else np.ones(224, np.float32))
                    w4g[4 * j + k, 0, q] = v[:WH]
                    w4g[4 * j + k, 1, q] = v[WH:]
        w4gs.append(np.ascontiguousarray(w4g))

    return w4gs, b4c, p2e, esb, rt, id16


def kernel(weight_logits, palette, image_h, image_w):
    weight_logits = np.asarray(weight_logits, np.float32)
    palette = np.asarray(palette, np.float32)
    assert int(image_h) == IMAGE_H and int(image_w) == IMAGE_W
    assert weight_logits.shape == (CANVAS_H, CANVAS_W, 3)

    if "nc" not in _CACHE:
        _CACHE["nc"] = _build_program()
    nc = _CACHE["nc"]

    from concourse import bass_utils

    w4gs, b4c, p2e, esb, rt, id16 = _host_consts(weight_logits, palette)
    in_maps = []
    for core in range(N_CORES):
        in_maps.append({
            "w4g_in": w4gs[core], "b4c_in": b4c, "p2e_in": p2e,
            "id16_in": id16, "esb_in": esb, "rt_in": rt,
        })
    res = bass_utils.run_bass_kernel_spmd(
        nc, in_maps, core_ids=list(range(N_CORES)))
    outs = [res.results[c]["out"] for c in range(N_CORES)]
    return np.concatenate(outs, axis=1)
